# revision 1
# baseline (speedup 1.0000x reference)
"""Trainium2 Bass kernel for nn_MessageProp (gnn_message_passing).

Reference computation (B=65536 rows, D=128, K=8 components, H=132 hidden):
    msgs  = einsum('kbd,ed->kbe', components, Wm) + bm   # message_map per component
    right = msgs.sum(0) @ Wu.T + bu                      # update_map
    x     = concat([signal, right], -1)
    h0 = relu(x @ W0.T + b0); h1 = relu(h0 @ W1.T + b1); h2 = relu(h1 @ W2.T + b2)
    out = h2 @ W3.T + b3

Key algebraic folds done on the host (all linear maps commute with the k-sum):
    csum = sum_k components[k]
    pre0 = signal @ A.T + csum @ Cm.T + b0'
      A   = W0[:, :D]
      Cm  = W0[:, D:] @ Wu @ Wm
      b0' = b0 + W0[:, D:] @ (Wu @ (K*bm) + bu)
so the device only computes csum (via SWDGE accumulate-DMA, zero compute) and a
4-matmul-layer MLP in feature-major layout (PE transposes at tile boundaries),
with float32r matmuls (1 cycle/row at N>=256 vs 4 for fp32).

Sharding: data-parallel over B across 8 cores (8192 rows each); weights replicated.
"""

import numpy as np
from contextlib import ExitStack, nullcontext

import concourse.bass as bass
import concourse.bacc as bacc
import concourse.tile as tile
import concourse.mybir as mybir
from concourse import bass_utils

F32 = mybir.dt.float32
R32 = mybir.dt.float32r
ACT = mybir.ActivationFunctionType

D = 128          # latent dim
H = 132          # FCBlock hidden width
B = 65536        # batch
K = 8            # components
NCORES = 8
RB = B // NCORES  # 8192 rows per core
TL = 2048        # rows per DMA load tile (1 MB per component slice)
M = TL // 128    # 16 row-chunks per partition within a load tile
NT = RB // TL    # 4 load tiles per core
SUB = 4          # m-blocks (128 rows each) per compute sub-tile -> 512 rows
NSUB = M // SUB  # 4 sub-tiles per load tile

# overlap knobs (HW-tuned via repeat-differencing; see module docstring)
ACC_MODE = "hw8"  # chain | pair | pair4 | hw8
ACC_SPLIT = 1    # independent accumulate chains per load tile (column split)
BUFS_LOADS = 3
BUFS_ACTS = 3
BUFS_OUT = 2
# tapered row-tile sizes (sum = RB); small final tiles shrink the drain tail
TILES = (1024,) * 7 + (512, 512)
# repeat whole body via HW loop (timing harness only; REPS>1 recomputes
# identical output on-device, isolating device time from RPC/transfer noise)
REPS = 1
# timing-only: skip all compute, just do the DMA pattern (output is garbage)
SKIP_COMPUTE = False
# timing-only: plain loads with no accumulate and no merge adds
PLAIN_LOADS = False
# timing-only: drop the 4-wide b-chunk path (wrong results; isolates PE load)
SKIP_B = False
# PSUM bank budget (8 total): ps_in*B_IN + ha*B_HA + hb*B_HB + po*B_PO + po2*B_PO2
B_IN = 2
B_HA = 3
B_HB = 1
B_PO = 1
B_PO2 = 1

# wpack column layout (all fp32, [128, NW]); see _build_wpack
_C_IDENT = 0
_C_W0A_SIG = 128
_C_W0A_CS = 256
_C_W1A_HI = 384
_C_W2A_HI = 512
_C_W3_HI = 640
_C_W1A_LO = 768    # [4,128] on partitions 0:4
_C_W2A_LO = 896    # [4,128]
_C_W3_LO = 1024    # [4,128]
_C_W0B_SIG = 1152  # [128,4]
_C_W0B_CS = 1156
_C_W1B_HI = 1160
_C_W2B_HI = 1164
_C_W1B_LO = 1168   # [4,4]
_C_W2B_LO = 1172
_C_B0A = 1176
_C_B1A = 1177
_C_B2A = 1178
_C_B3 = 1179
_C_B0B = 1180      # [4,1]
_C_B1B = 1181
_C_B2B = 1182
NW = 1184


def _build_wpack(Wm, bm, Wu, bu, W0, b0, W1, b1, W2, b2, W3, b3):
    f8 = np.float64
    Wm, bm, Wu, bu = Wm.astype(f8), bm.astype(f8), Wu.astype(f8), bu.astype(f8)
    W0, b0, W1, b1 = W0.astype(f8), b0.astype(f8), W1.astype(f8), b1.astype(f8)
    W2, b2, W3, b3 = W2.astype(f8), b2.astype(f8), W3.astype(f8), b3.astype(f8)

    A = W0[:, :D]                              # [H, D]
    W0r = W0[:, D:]                            # [H, D]
    Cm = W0r @ (Wu @ Wm)                       # [H, D]
    b0p = b0 + W0r @ (Wu @ (K * bm) + bu)      # [H]

    w = np.zeros((128, NW), dtype=np.float64)
    w[:, _C_IDENT:_C_IDENT + 128] = np.eye(128)
    # L0: lhsT[p=d, m=h] = A.T / Cm.T
    w[:, _C_W0A_SIG:_C_W0A_SIG + 128] = A.T[:, :128]
    w[:, _C_W0A_CS:_C_W0A_CS + 128] = Cm.T[:, :128]
    w[:, _C_W0B_SIG:_C_W0B_SIG + 4] = A.T[:, 128:]
    w[:, _C_W0B_CS:_C_W0B_CS + 4] = Cm.T[:, 128:]
    # L1/L2: lhsT[p=h_in, m=h_out] = Wx.T
    for Wx, chi, clo, cbhi, cblo in (
        (W1, _C_W1A_HI, _C_W1A_LO, _C_W1B_HI, _C_W1B_LO),
        (W2, _C_W2A_HI, _C_W2A_LO, _C_W2B_HI, _C_W2B_LO),
    ):
        WT = Wx.T                              # [132 in, 132 out]
        w[:, chi:chi + 128] = WT[:128, :128]
        w[:4, clo:clo + 128] = WT[128:, :128]
        w[:, cbhi:cbhi + 4] = WT[:128, 128:]
        w[:4, cblo:cblo + 4] = WT[128:, 128:]
    # L3: lhsT[p=h2, m=d] = W3.T
    W3T = W3.T                                 # [132, 128]
    w[:, _C_W3_HI:_C_W3_HI + 128] = W3T[:128, :]
    w[:4, _C_W3_LO:_C_W3_LO + 128] = W3T[128:, :]
    # biases
    w[:, _C_B0A] = b0p[:128]
    w[:, _C_B1A] = b1[:128]
    w[:, _C_B2A] = b2[:128]
    w[:, _C_B3] = b3
    w[:4, _C_B0B] = b0p[128:]
    w[:4, _C_B1B] = b1[128:]
    w[:4, _C_B2B] = b2[128:]
    return np.ascontiguousarray(w, dtype=np.float32)


def _trace_kernel(nc: bass.Bass):
    assert sum(TILES) == RB and all(tl % (SUB * 128) == 0 for tl in TILES)
    sig = nc.dram_tensor("sig", [RB, D], R32, kind="ExternalInput")
    comp = nc.dram_tensor("comp", [K, RB, D], F32, kind="ExternalInput")
    wpack = nc.dram_tensor("wpack", [128, NW], F32, kind="ExternalInput")
    wpackr = nc.dram_tensor("wpackr", [128, NW], R32, kind="ExternalInput")
    out = nc.dram_tensor("out", [RB, D], F32, kind="ExternalOutput")

    # per-tile views; within tile t: row = r0 + p*M_t + m, free layout (m d)
    def tile_views(r0, tl):
        m = tl // 128
        s_v = sig.ap()[r0:r0 + tl, :].rearrange("(p m) d -> p (m d)", p=128, m=m)
        c_v = [comp.ap()[k, r0:r0 + tl, :].rearrange("(p m) d -> p (m d)", p=128, m=m)
               for k in range(K)]
        o_v = out.ap()[r0:r0 + tl, :].rearrange("(p m) d -> p (m d)", p=128, m=m)
        return s_v, c_v, o_v

    with tile.TileContext(nc) as tc, ExitStack() as ctx:
        wpool = ctx.enter_context(tc.tile_pool(name="weights", bufs=1))
        loads = ctx.enter_context(tc.tile_pool(name="loads", bufs=BUFS_LOADS))
        acts = ctx.enter_context(tc.tile_pool(name="acts", bufs=BUFS_ACTS))
        opool = ctx.enter_context(tc.tile_pool(name="outs", bufs=BUFS_OUT))
        psum = ctx.enter_context(tc.tile_pool(name="psum", bufs=2, space="PSUM"))

        wsb = wpool.tile([128, NW], F32)
        nc.sync.dma_start(wsb[:], wpack.ap())
        wsr = wpool.tile([128, NW], R32)
        nc.sync.dma_start(wsr[:], wpackr.ap())

        ident = wsb[:, _C_IDENT:_C_IDENT + 128]
        identr = wsr[:, _C_IDENT:_C_IDENT + 128]

        def wcol(c, n=128, parts=128):
            return wsb[:parts, c:c + n]

        def wcolr(c, n=128, parts=128):
            return wsr[:parts, c:c + n]

        with (tc.For_i(0, REPS, 1) if REPS > 1 else nullcontext()):
            r0 = 0
            for t, TLt in enumerate(TILES):
                NSUB = TLt // (SUB * 128)
                sig_v, comp_v, out_v = tile_views(r0, TLt)
                r0 += TLt
                sig_nat = loads.tile([128, TLt], R32, tag="sig_nat")
                nc.scalar.dma_start(sig_nat[:], sig_v)

                cs_nat = loads.tile([128, TLt], F32, tag="cs_nat")
                CW = TLt // ACC_SPLIT
                if PLAIN_LOADS:
                    lands = [cs_nat]
                    for i in range(1, K):
                        ld = loads.tile([128, TLt], F32, tag=f"cs{i}")
                        lands.append(ld)
                    for i in range(K):
                        eng = nc.sync if i % 2 == 0 else nc.scalar
                        eng.dma_start(lands[i][:], comp_v[i])
                elif ACC_MODE == "pair":
                    cs_nat2 = loads.tile([128, TLt], F32, tag="cs_nat2")
                    for h in range(ACC_SPLIT):
                        cl = slice(h * CW, (h + 1) * CW)
                        nc.gpsimd.dma_start(cs_nat[:, cl], comp_v[0][:, cl])
                        nc.gpsimd.dma_start(cs_nat2[:, cl], comp_v[1][:, cl])
                        for k in range(2, K, 2):
                            nc.gpsimd.dma_start(cs_nat[:, cl], comp_v[k][:, cl],
                                                accum_op=mybir.AluOpType.add)
                            nc.gpsimd.dma_start(cs_nat2[:, cl], comp_v[k + 1][:, cl],
                                                accum_op=mybir.AluOpType.add)
                    cs_sum = loads.tile([128, TLt], R32, tag="cs_sum")
                    nc.vector.tensor_add(cs_sum[:], cs_nat[:], cs_nat2[:])
                elif ACC_MODE == "pair4":
                    # 4 SWDGE chains of depth 2, then a DVE/Pool merge tree
                    cs2 = loads.tile([128, TLt], F32, tag="cs2")
                    cs3 = loads.tile([128, TLt], F32, tag="cs3")
                    cs4 = loads.tile([128, TLt], F32, tag="cs4")
                    for i, dst in enumerate((cs_nat, cs2, cs3, cs4)):
                        nc.gpsimd.dma_start(dst[:], comp_v[i])
                        nc.gpsimd.dma_start(dst[:], comp_v[i + 4],
                                            accum_op=mybir.AluOpType.add)
                    nc.vector.tensor_add(cs2[:], cs2[:], cs_nat[:])
                    nc.gpsimd.tensor_add(cs3[:], cs3[:], cs4[:])
                    cs_sum = loads.tile([128, TLt], R32, tag="cs_sum")
                    nc.vector.tensor_add(cs_sum[:], cs2[:], cs3[:])
                elif ACC_MODE == "hw8":
                    # 8 fully parallel HWDGE loads + merge tree on DVE/Pool
                    lands = [cs_nat]
                    for i in range(1, K):
                        ld = loads.tile([128, TLt], F32, tag=f"cs{i}")
                        lands.append(ld)
                    for i in range(K):
                        eng = nc.sync if i % 2 == 0 else nc.scalar
                        eng.dma_start(lands[i][:], comp_v[i])
                    nc.vector.tensor_add(lands[1][:], lands[1][:], lands[0][:])
                    nc.gpsimd.tensor_add(lands[3][:], lands[3][:], lands[2][:])
                    nc.vector.tensor_add(lands[5][:], lands[5][:], lands[4][:])
                    nc.gpsimd.tensor_add(lands[7][:], lands[7][:], lands[6][:])
                    nc.vector.tensor_add(lands[1][:], lands[1][:], lands[3][:])
                    nc.gpsimd.tensor_add(lands[5][:], lands[5][:], lands[7][:])
                    cs_sum = loads.tile([128, TLt], R32, tag="cs_sum")
                    nc.vector.tensor_add(cs_sum[:], lands[1][:], lands[5][:])
                else:
                    for h in range(ACC_SPLIT):
                        cl = slice(h * CW, (h + 1) * CW)
                        nc.gpsimd.dma_start(cs_nat[:, cl], comp_v[0][:, cl])
                        for k in range(1, K):
                            nc.gpsimd.dma_start(cs_nat[:, cl], comp_v[k][:, cl],
                                                accum_op=mybir.AluOpType.add)
                    cs_sum = loads.tile([128, TLt], R32, tag="cs_sum")
                    nc.vector.tensor_copy(cs_sum[:], cs_nat[:])

                out_sb = opool.tile([128, TLt], F32, tag="out_sb")

                if SKIP_COMPUTE:
                    nc.vector.tensor_copy(out_sb[:], sig_nat[:])
                    nc.sync.dma_start(out_v, out_sb[:])
                    continue

                for s in range(NSUB):
                    cols = slice(s * SUB * 128, (s + 1) * SUB * 128)

                    # ---- transpose signal + csum blocks into feature-major ----
                    ps_sig = psum.tile([128, SUB * 128], R32, tag="ps_in", bufs=B_IN)
                    for j in range(SUB):
                        mb = (s * SUB + j) * 128
                        nc.tensor.transpose(ps_sig[:, j * 128:(j + 1) * 128],
                                            sig_nat[:, mb:mb + 128], identr)
                    sigT = acts.tile([128, SUB * 128], R32, tag="sigT")
                    nc.scalar.activation(sigT[:], ps_sig[:], ACT.Copy)

                    ps_cs = psum.tile([128, SUB * 128], R32, tag="ps_in", bufs=B_IN)
                    for j in range(SUB):
                        mb = (s * SUB + j) * 128
                        nc.tensor.transpose(ps_cs[:, j * 128:(j + 1) * 128],
                                            cs_sum[:, mb:mb + 128], identr)
                    csT = acts.tile([128, SUB * 128], R32, tag="csT")
                    nc.vector.tensor_copy(csT[:], ps_cs[:])

                    sigT_r = sigT[:]
                    csT_r = csT[:]

                    # ---- L0: h0 = relu(A@sigT + Cm@csT + b0') ----
                    ps_h0a = psum.tile([128, SUB * 128], F32, tag="ha", bufs=B_HA)
                    nc.tensor.matmul(ps_h0a[:], wcolr(_C_W0A_SIG),
                                     sigT[:], start=True, stop=False)
                    nc.tensor.matmul(ps_h0a[:], wcolr(_C_W0A_CS),
                                     csT[:], start=False, stop=True)
                    if not SKIP_B:
                        ps_h0b = psum.tile([4, SUB * 128], F32, tag="hb", bufs=B_HB)
                        nc.tensor.matmul(ps_h0b[:], wcolr(_C_W0B_SIG, 4),
                                         sigT[:], start=True, stop=False)
                        nc.tensor.matmul(ps_h0b[:], wcolr(_C_W0B_CS, 4),
                                         csT[:], start=False, stop=True)
                    h0a = acts.tile([128, SUB * 128], R32, tag="h0a")
                    nc.vector.tensor_scalar(h0a[:], ps_h0a[:],
                                            wcol(_C_B0A, 1), 0.0,
                                            mybir.AluOpType.add,
                                            mybir.AluOpType.max)
                    if not SKIP_B:
                        h0b = acts.tile([4, SUB * 128], R32, tag="h0b")
                        nc.scalar.activation(h0b[:], ps_h0b[:], ACT.Relu,
                                             bias=wcol(_C_B0B, 1, parts=4))

                    # ---- L1 ----
                    ps_h1a = psum.tile([128, SUB * 128], F32, tag="ha", bufs=B_HA)
                    nc.tensor.matmul(ps_h1a[:], wcolr(_C_W1A_HI),
                                     h0a[:], start=True, stop=SKIP_B)
                    if not SKIP_B:
                        nc.tensor.matmul(ps_h1a[:], wcolr(_C_W1A_LO, 128, parts=4),
                                         h0b[:], start=False, stop=True)
                    if not SKIP_B:
                        ps_h1b = psum.tile([4, SUB * 128], F32, tag="hb", bufs=B_HB)
                        nc.tensor.matmul(ps_h1b[:], wcolr(_C_W1B_HI, 4),
                                         h0a[:], start=True, stop=False)
                        nc.tensor.matmul(ps_h1b[:], wcolr(_C_W1B_LO, 4, parts=4),
                                         h0b[:], start=False, stop=True)
                    h1a = acts.tile([128, SUB * 128], R32, tag="h1a")
                    nc.vector.tensor_scalar(h1a[:], ps_h1a[:],
                                            wcol(_C_B1A, 1), 0.0,
                                            mybir.AluOpType.add,
                                            mybir.AluOpType.max)
                    if not SKIP_B:
                        h1b = acts.tile([4, SUB * 128], R32, tag="h1b")
                        nc.scalar.activation(h1b[:], ps_h1b[:], ACT.Relu,
                                             bias=wcol(_C_B1B, 1, parts=4))

                    # ---- L2 ----
                    ps_h2a = psum.tile([128, SUB * 128], F32, tag="ha", bufs=B_HA)
                    nc.tensor.matmul(ps_h2a[:], wcolr(_C_W2A_HI),
                                     h1a[:], start=True, stop=SKIP_B)
                    if not SKIP_B:
                        nc.tensor.matmul(ps_h2a[:], wcolr(_C_W2A_LO, 128, parts=4),
                                         h1b[:], start=False, stop=True)
                    if not SKIP_B:
                        ps_h2b = psum.tile([4, SUB * 128], F32, tag="hb", bufs=B_HB)
                        nc.tensor.matmul(ps_h2b[:], wcolr(_C_W2B_HI, 4),
                                         h1a[:], start=True, stop=False)
                        nc.tensor.matmul(ps_h2b[:], wcolr(_C_W2B_LO, 4, parts=4),
                                         h1b[:], start=False, stop=True)
                    h2a = acts.tile([128, SUB * 128], R32, tag="h2a")
                    nc.scalar.activation(h2a[:], ps_h2a[:], ACT.Relu,
                                         bias=wcol(_C_B2A, 1))
                    if not SKIP_B:
                        h2b = acts.tile([4, SUB * 128], R32, tag="h2b")
                        nc.scalar.activation(h2b[:], ps_h2b[:], ACT.Relu,
                                             bias=wcol(_C_B2B, 1, parts=4))

                    # ---- L3: outT = W3 @ h2 + b3 (feature-major) ----
                    ps_oT = psum.tile([128, SUB * 128], F32, tag="po", bufs=B_PO)
                    nc.tensor.matmul(ps_oT[:], wcolr(_C_W3_HI),
                                     h2a[:], start=True, stop=SKIP_B)
                    if not SKIP_B:
                        nc.tensor.matmul(ps_oT[:], wcolr(_C_W3_LO, 128, parts=4),
                                         h2b[:], start=False, stop=True)
                    oT = acts.tile([128, SUB * 128], F32, tag="oT")
                    nc.scalar.activation(oT[:], ps_oT[:], ACT.Identity,
                                         bias=wcol(_C_B3, 1))

                    # ---- transpose back to row-major and stage the store ----
                    ps_on = psum.tile([128, SUB * 128], F32, tag="po2", bufs=B_PO2)
                    for j in range(SUB):
                        nc.tensor.transpose(ps_on[:, j * 128:(j + 1) * 128],
                                            oT[:, j * 128:(j + 1) * 128], ident)
                    nc.vector.tensor_copy(out_sb[:, cols], ps_on[:])

                nc.sync.dma_start(out_v, out_sb[:])

    return nc


_CACHED_NC = None


def _get_nc():
    global _CACHED_NC
    if _CACHED_NC is None:
        nc = bacc.Bacc("TRN2", target_bir_lowering=False, debug=False,
                       enable_asserts=False, num_devices=NCORES)
        _trace_kernel(nc)
        nc.compile()
        _CACHED_NC = nc
    return _CACHED_NC


def _run(signal, components, Wm, bm, Wu, bu, W0, b0, W1, b1, W2, b2, W3, b3,
         **spmd_kwargs):
    signal = np.ascontiguousarray(np.asarray(signal, dtype=np.float32))
    components = np.ascontiguousarray(np.asarray(components, dtype=np.float32))
    wpack = _build_wpack(*[np.asarray(a, dtype=np.float32) for a in
                           (Wm, bm, Wu, bu, W0, b0, W1, b1, W2, b2, W3, b3)])

    nc = _get_nc()
    in_maps = []
    for c in range(NCORES):
        r0 = c * RB
        in_maps.append({
            "sig": signal[r0:r0 + RB],
            "comp": np.ascontiguousarray(components[:, r0:r0 + RB, :]),
            "wpack": wpack,
            "wpackr": wpack,
        })
    return bass_utils.run_bass_kernel_spmd(nc, in_maps,
                                           core_ids=list(range(NCORES)),
                                           **spmd_kwargs)


def kernel(**inputs):
    res = _run(**inputs)
    return np.concatenate([res.results[c]["out"] for c in range(NCORES)], axis=0)



# revision 6
# speedup vs baseline: 1.2728x; 1.2728x over previous
"""Trainium2 Bass kernel for nn_MessageProp (gnn_message_passing).

Reference computation (B=65536 rows, D=128, K=8 components, H=132 hidden):
    msgs  = einsum('kbd,ed->kbe', components, Wm) + bm   # message_map per component
    right = msgs.sum(0) @ Wu.T + bu                      # update_map
    x     = concat([signal, right], -1)
    h0 = relu(x @ W0.T + b0); h1 = relu(h0 @ W1.T + b1); h2 = relu(h1 @ W2.T + b2)
    out = h2 @ W3.T + b3

Algebraic folds done on the host (all linear maps commute with the k-sum):
    csum = sum_k components[k]
    pre0 = A @ sigT + Cm @ csT + b0'
      A   = W0[:, :D]
      Cm  = W0[:, D:] @ Wu @ Wm
      b0' = b0 + W0[:, D:] @ (Wu @ (K*bm) + bu)

Layout strategy (v2): everything is staged FEATURE-MAJOR in HBM by the host
(sigT [D,RB], compT [K,D,RB], outT [D,RB]) and in bf16.  This
 * halves HBM traffic (the kernel is memory-bound: 20MB/core vs 40MB fp32),
 * eliminates every PE transpose (the fp32 row-major design spent 12 PE
   transpose passes per 512-row subtile),
 * runs all matmuls at 1 cycle/row (bf16) with fp32 PSUM accumulation,
 * lets DVE merge-adds hit the 2-byte fast path.
The device computes csum via a parallel-load + add-tree, then a 4-layer
feature-major MLP (weights stationary, batch moving), H=132 split 128+4.

Sharding: data-parallel over B across 8 cores (8192 rows each); weights
replicated. Host does the (free) layout transposes + dtype casts; device
output outT is transposed back and upcast on the host.
"""

import numpy as np
import ml_dtypes
from contextlib import ExitStack, nullcontext

import concourse.bass as bass
import concourse.bacc as bacc
import concourse.tile as tile
import concourse.mybir as mybir
from concourse import bass_utils

F32 = mybir.dt.float32
BF16 = mybir.dt.bfloat16
ACT = mybir.ActivationFunctionType
ADD = mybir.AluOpType.add
MAX = mybir.AluOpType.max
NPBF = ml_dtypes.bfloat16

D = 128          # latent dim
H = 132          # FCBlock hidden width
B = 65536        # batch
K = 8            # components
NCORES = 8
RB = B // NCORES  # 8192 rows per core
FREE = 512       # rows per compute sub-tile (PSUM bank = 512 f32)

# row-tile sizes (sum = RB, all multiples of FREE); small final tiles shrink
# the pipeline drain tail
TILES = (2048, 2048, 2048, 1024, 512, 512)
BUFS_LOADS = 2
BUFS_ACTS = 3
BUFS_OUT = 2
# PSUM bank budget (8 total)
B_HA = 4
B_HB = 2
B_PO = 2
# merge-tree engine per add: 'v' = vector (DVE), 'p' = gpsimd (Pool)
# adds: l1+=l0, l3+=l2, l5+=l4, l7+=l6, l1+=l3, l5+=l7, cs=l1+l5
# (Pool cannot access PSUM, so it only ever gets these SBUF-only adds)
MERGE_ENGS = "ppppvvv"
# DMA issue engines for the 8 component loads (cycled), the signal load and
# the output store: 's' = sync (SP), 'a' = scalar (Act), 'v' = vector,
# 'p' = gpsimd
COMP_DMA_ENGS = "sasasasa"
SIG_DMA_ENG = "s"
OUT_DMA_ENG = "s"
# engines for the per-layer bias+relu (psum -> sbuf) ops; must be 'v' or 'a'
ENG_H0A = "v"
ENG_H1A = "v"
ENG_H2A = "v"
ENG_OT = "a"
ENG_H0B = "a"
ENG_H1B = "a"
ENG_H2B = "a"
# timing-only: skip all compute, just do the DMA pattern (output is garbage)
SKIP_COMPUTE = False
# repeat whole body via HW loop (timing harness only)
REPS = 1

# bf16 weight-pack column layout ([128, NWB]); lhsT tensors, K on partitions
_C_W0A_SIG = 0      # [128,128] A.T[:, :128]
_C_W0A_CS = 128     # [128,128] Cm.T[:, :128]
_C_W1A_HI = 256     # [128,128] W1.T[:128, :128]
_C_W2A_HI = 384
_C_W3_HI = 512      # [128,128] W3.T[:128, :]
_C_W1A_LO = 640     # [4,128]  W1.T[128:, :128] on partitions 0:4
_C_W2A_LO = 768
_C_W3_LO = 896      # [4,128]  W3.T[128:, :]
_C_W0B_SIG = 1024   # [128,4]  A.T[:, 128:]
_C_W0B_CS = 1028
_C_W1B_HI = 1032    # [128,4]  W1.T[:128, 128:]
_C_W2B_HI = 1036
_C_W1B_LO = 1040    # [4,4]    W1.T[128:, 128:]
_C_W2B_LO = 1044
NWB = 1048
# f32 bias-pack column layout ([128, NWF])
_C_B0A = 0
_C_B1A = 1
_C_B2A = 2
_C_B3 = 3
_C_B0B = 4          # [4,1]
_C_B1B = 5
_C_B2B = 6
NWF = 8


def _build_wpacks(Wm, bm, Wu, bu, W0, b0, W1, b1, W2, b2, W3, b3):
    f8 = np.float64
    Wm, bm, Wu, bu = Wm.astype(f8), bm.astype(f8), Wu.astype(f8), bu.astype(f8)
    W0, b0, W1, b1 = W0.astype(f8), b0.astype(f8), W1.astype(f8), b1.astype(f8)
    W2, b2, W3, b3 = W2.astype(f8), b2.astype(f8), W3.astype(f8), b3.astype(f8)

    A = W0[:, :D]                              # [H, D]
    W0r = W0[:, D:]                            # [H, D]
    Cm = W0r @ (Wu @ Wm)                       # [H, D]
    b0p = b0 + W0r @ (Wu @ (K * bm) + bu)      # [H]

    wb = np.zeros((128, NWB), dtype=np.float64)
    wb[:, _C_W0A_SIG:_C_W0A_SIG + 128] = A.T[:, :128]
    wb[:, _C_W0A_CS:_C_W0A_CS + 128] = Cm.T[:, :128]
    wb[:, _C_W0B_SIG:_C_W0B_SIG + 4] = A.T[:, 128:]
    wb[:, _C_W0B_CS:_C_W0B_CS + 4] = Cm.T[:, 128:]
    for Wx, chi, clo, cbhi, cblo in (
        (W1, _C_W1A_HI, _C_W1A_LO, _C_W1B_HI, _C_W1B_LO),
        (W2, _C_W2A_HI, _C_W2A_LO, _C_W2B_HI, _C_W2B_LO),
    ):
        WT = Wx.T                              # [132 in, 132 out]
        wb[:, chi:chi + 128] = WT[:128, :128]
        wb[:4, clo:clo + 128] = WT[128:, :128]
        wb[:, cbhi:cbhi + 4] = WT[:128, 128:]
        wb[:4, cblo:cblo + 4] = WT[128:, 128:]
    W3T = W3.T                                 # [132, 128]
    wb[:, _C_W3_HI:_C_W3_HI + 128] = W3T[:128, :]
    wb[:4, _C_W3_LO:_C_W3_LO + 128] = W3T[128:, :]

    wf = np.zeros((128, NWF), dtype=np.float64)
    wf[:, _C_B0A] = b0p[:128]
    wf[:, _C_B1A] = b1[:128]
    wf[:, _C_B2A] = b2[:128]
    wf[:, _C_B3] = b3
    wf[:4, _C_B0B] = b0p[128:]
    wf[:4, _C_B1B] = b1[128:]
    wf[:4, _C_B2B] = b2[128:]
    return (np.ascontiguousarray(wb, dtype=NPBF),
            np.ascontiguousarray(wf, dtype=np.float32))


def _trace_kernel(nc: bass.Bass):
    assert sum(TILES) == RB and all(t % FREE == 0 for t in TILES)
    sigT = nc.dram_tensor("sigT", [D, RB], BF16, kind="ExternalInput")
    compT = nc.dram_tensor("compT", [K, D, RB], BF16, kind="ExternalInput")
    wbd = nc.dram_tensor("wb", [128, NWB], BF16, kind="ExternalInput")
    wfd = nc.dram_tensor("wf", [128, NWF], F32, kind="ExternalInput")
    outT = nc.dram_tensor("outT", [D, RB], BF16, kind="ExternalOutput")

    with tile.TileContext(nc) as tc, ExitStack() as ctx:
        wpool = ctx.enter_context(tc.tile_pool(name="weights", bufs=1))
        loads = ctx.enter_context(tc.tile_pool(name="loads", bufs=BUFS_LOADS))
        acts = ctx.enter_context(tc.tile_pool(name="acts", bufs=BUFS_ACTS))
        opool = ctx.enter_context(tc.tile_pool(name="outs", bufs=BUFS_OUT))
        psum = ctx.enter_context(tc.tile_pool(name="psum", bufs=2, space="PSUM"))

        eng = {"s": nc.sync, "a": nc.scalar, "v": nc.vector, "p": nc.gpsimd}

        wbs = wpool.tile([128, NWB], BF16)
        nc.sync.dma_start(wbs[:], wbd.ap())
        wfs = wpool.tile([128, NWF], F32)
        nc.sync.dma_start(wfs[:], wfd.ap())

        def wcol(c, n=128, parts=128):
            return wbs[:parts, c:c + n]

        def bcol(c, parts=128):
            return wfs[:parts, c:c + 1]

        def relu_bias(e, out, ps, bc, parts=128):
            if e == "a":
                nc.scalar.activation(out, ps, ACT.Relu, bias=bcol(bc, parts))
            else:
                eng[e].tensor_scalar(out, ps, bcol(bc, parts), 0.0, ADD, MAX)

        with (tc.For_i(0, REPS, 1) if REPS > 1 else nullcontext()):
            r0 = 0
            for t, TLt in enumerate(TILES):
                sig_v = sigT.ap()[:, r0:r0 + TLt]
                comp_v = [compT.ap()[k, :, r0:r0 + TLt] for k in range(K)]
                out_v = outT.ap()[:, r0:r0 + TLt]
                r0 += TLt

                sig_t = loads.tile([128, TLt], BF16, tag="sig")
                eng[SIG_DMA_ENG].dma_start(sig_t[:], sig_v)

                lands = [loads.tile([128, TLt], BF16, tag=f"cs{i}",
                                    name=f"cs{i}") for i in range(K)]
                for i in range(K):
                    eng[COMP_DMA_ENGS[i % len(COMP_DMA_ENGS)]].dma_start(
                        lands[i][:], comp_v[i])
                cs_sum = loads.tile([128, TLt], BF16, tag="cs_sum")
                me = [eng[c] for c in MERGE_ENGS]
                me[0].tensor_add(lands[1][:], lands[1][:], lands[0][:])
                me[1].tensor_add(lands[3][:], lands[3][:], lands[2][:])
                me[2].tensor_add(lands[5][:], lands[5][:], lands[4][:])
                me[3].tensor_add(lands[7][:], lands[7][:], lands[6][:])
                me[4].tensor_add(lands[1][:], lands[1][:], lands[3][:])
                me[5].tensor_add(lands[5][:], lands[5][:], lands[7][:])
                me[6].tensor_add(cs_sum[:], lands[1][:], lands[5][:])

                out_sb = opool.tile([128, TLt], BF16, tag="out_sb")

                if SKIP_COMPUTE:
                    nc.vector.tensor_copy(out_sb[:], sig_t[:])
                    eng[OUT_DMA_ENG].dma_start(out_v, out_sb[:])
                    continue

                for s in range(TLt // FREE):
                    cols = slice(s * FREE, (s + 1) * FREE)

                    # ---- L0: h0 = relu(A@sigT + Cm@csT + b0') ----
                    ps_h0a = psum.tile([128, FREE], F32, tag="ha", bufs=B_HA)
                    nc.tensor.matmul(ps_h0a[:], wcol(_C_W0A_SIG),
                                     sig_t[:, cols], start=True, stop=False)
                    nc.tensor.matmul(ps_h0a[:], wcol(_C_W0A_CS),
                                     cs_sum[:, cols], start=False, stop=True)
                    ps_h0b = psum.tile([4, FREE], F32, tag="hb", bufs=B_HB)
                    nc.tensor.matmul(ps_h0b[:], wcol(_C_W0B_SIG, 4),
                                     sig_t[:, cols], start=True, stop=False)
                    nc.tensor.matmul(ps_h0b[:], wcol(_C_W0B_CS, 4),
                                     cs_sum[:, cols], start=False, stop=True)
                    h0a = acts.tile([128, FREE], BF16, tag="h0a")
                    relu_bias(ENG_H0A, h0a[:], ps_h0a[:], _C_B0A)
                    h0b = acts.tile([4, FREE], BF16, tag="h0b")
                    relu_bias(ENG_H0B, h0b[:], ps_h0b[:], _C_B0B, parts=4)

                    # ---- L1 ----
                    ps_h1a = psum.tile([128, FREE], F32, tag="ha", bufs=B_HA)
                    nc.tensor.matmul(ps_h1a[:], wcol(_C_W1A_HI),
                                     h0a[:], start=True, stop=False)
                    nc.tensor.matmul(ps_h1a[:], wcol(_C_W1A_LO, 128, parts=4),
                                     h0b[:], start=False, stop=True)
                    ps_h1b = psum.tile([4, FREE], F32, tag="hb", bufs=B_HB)
                    nc.tensor.matmul(ps_h1b[:], wcol(_C_W1B_HI, 4),
                                     h0a[:], start=True, stop=False)
                    nc.tensor.matmul(ps_h1b[:], wcol(_C_W1B_LO, 4, parts=4),
                                     h0b[:], start=False, stop=True)
                    h1a = acts.tile([128, FREE], BF16, tag="h1a")
                    relu_bias(ENG_H1A, h1a[:], ps_h1a[:], _C_B1A)
                    h1b = acts.tile([4, FREE], BF16, tag="h1b")
                    relu_bias(ENG_H1B, h1b[:], ps_h1b[:], _C_B1B, parts=4)

                    # ---- L2 ----
                    ps_h2a = psum.tile([128, FREE], F32, tag="ha", bufs=B_HA)
                    nc.tensor.matmul(ps_h2a[:], wcol(_C_W2A_HI),
                                     h1a[:], start=True, stop=False)
                    nc.tensor.matmul(ps_h2a[:], wcol(_C_W2A_LO, 128, parts=4),
                                     h1b[:], start=False, stop=True)
                    ps_h2b = psum.tile([4, FREE], F32, tag="hb", bufs=B_HB)
                    nc.tensor.matmul(ps_h2b[:], wcol(_C_W2B_HI, 4),
                                     h1a[:], start=True, stop=False)
                    nc.tensor.matmul(ps_h2b[:], wcol(_C_W2B_LO, 4, parts=4),
                                     h1b[:], start=False, stop=True)
                    h2a = acts.tile([128, FREE], BF16, tag="h2a")
                    relu_bias(ENG_H2A, h2a[:], ps_h2a[:], _C_B2A)
                    h2b = acts.tile([4, FREE], BF16, tag="h2b")
                    relu_bias(ENG_H2B, h2b[:], ps_h2b[:], _C_B2B, parts=4)

                    # ---- L3: outT = W3 @ h2 + b3 (feature-major) ----
                    ps_o = psum.tile([128, FREE], F32, tag="po", bufs=B_PO)
                    nc.tensor.matmul(ps_o[:], wcol(_C_W3_HI),
                                     h2a[:], start=True, stop=False)
                    nc.tensor.matmul(ps_o[:], wcol(_C_W3_LO, 128, parts=4),
                                     h2b[:], start=False, stop=True)
                    eng[ENG_OT].activation(out_sb[:, cols], ps_o[:],
                                           ACT.Identity, bias=bcol(_C_B3))

                eng[OUT_DMA_ENG].dma_start(out_v, out_sb[:])

    return nc


_CACHED_NC = None


def _get_nc():
    global _CACHED_NC
    if _CACHED_NC is None:
        nc = bacc.Bacc("TRN2", target_bir_lowering=False, debug=False,
                       enable_asserts=False, num_devices=NCORES)
        _trace_kernel(nc)
        nc.compile()
        _CACHED_NC = nc
    return _CACHED_NC


def make_in_maps(inputs):
    """Host staging: shard over B, transpose to feature-major, cast to bf16."""
    signal = np.asarray(inputs["signal"], dtype=np.float32)
    components = np.asarray(inputs["components"], dtype=np.float32)
    wb, wf = _build_wpacks(*[np.asarray(inputs[k], dtype=np.float32) for k in
                             ("Wm", "bm", "Wu", "bu", "W0", "b0",
                              "W1", "b1", "W2", "b2", "W3", "b3")])
    sigT = signal.astype(NPBF).T                      # [D, B] view
    compT = components.astype(NPBF).transpose(0, 2, 1)  # [K, D, B] view
    in_maps = []
    for c in range(NCORES):
        r0 = c * RB
        in_maps.append({
            "sigT": np.ascontiguousarray(sigT[:, r0:r0 + RB]),
            "compT": np.ascontiguousarray(compT[:, :, r0:r0 + RB]),
            "wb": wb,
            "wf": wf,
        })
    return in_maps


def kernel(**inputs):
    nc = _get_nc()
    res = bass_utils.run_bass_kernel_spmd(nc, make_in_maps(inputs),
                                          core_ids=list(range(NCORES)))
    out = np.concatenate(
        [res.results[c]["outT"].astype(np.float32).T for c in range(NCORES)],
        axis=0)
    return np.ascontiguousarray(out)


# revision 9
# speedup vs baseline: 2.3046x; 1.8107x over previous
"""Trainium2 Bass kernel for nn_MessageProp (gnn_message_passing).

Reference computation (B=65536 rows, D=128, K=8 components, H=132 hidden):
    msgs  = einsum('kbd,ed->kbe', components, Wm) + bm   # message_map per component
    right = msgs.sum(0) @ Wu.T + bu                      # update_map
    x     = concat([signal, right], -1)
    h0 = relu(x @ W0.T + b0); h1 = relu(h0 @ W1.T + b1); h2 = relu(h1 @ W2.T + b2)
    out = h2 @ W3.T + b3

Algebraic folds done on the host (all linear maps commute with the k-sum):
    csum = sum_k components[k]
    pre0 = A @ sigT + Cm @ csT + b0'
      A   = W0[:, :D]
      Cm  = W0[:, D:] @ Wu @ Wm
      b0' = b0 + W0[:, D:] @ (Wu @ (K*bm) + bu)

Layout strategy (v2): everything is staged FEATURE-MAJOR in HBM by the host
(sigT [D,RB], compT [K,D,RB], outT [D,RB]) and in bf16.  This
 * halves HBM traffic (the kernel is memory-bound: 20MB/core vs 40MB fp32),
 * eliminates every PE transpose (the fp32 row-major design spent 12 PE
   transpose passes per 512-row subtile),
 * runs all matmuls at 1 cycle/row (bf16) with fp32 PSUM accumulation,
 * lets DVE merge-adds hit the 2-byte fast path.
The device computes csum via a parallel-load + add-tree, then a 4-layer
feature-major MLP (weights stationary, batch moving), H=132 split 128+4.

Sharding: data-parallel over B across 8 cores (8192 rows each); weights
replicated. Host does the (free) layout transposes + dtype casts; device
output outT is transposed back and upcast on the host.
"""

import numpy as np
import ml_dtypes
from contextlib import ExitStack, nullcontext

import concourse.bass as bass
import concourse.bacc as bacc
import concourse.tile as tile
import concourse.mybir as mybir
from concourse import bass_utils

F32 = mybir.dt.float32
BF16 = mybir.dt.bfloat16
ACT = mybir.ActivationFunctionType
ADD = mybir.AluOpType.add
MAX = mybir.AluOpType.max
NPBF = ml_dtypes.bfloat16

D = 128          # latent dim
H = 132          # FCBlock hidden width
B = 65536        # batch
K = 8            # components
NCORES = 8
RB = B // NCORES  # 8192 rows per core
FREE = 512       # rows per compute sub-tile (PSUM bank = 512 f32)

# row-tile sizes (sum = RB, all multiples of FREE); small final tiles shrink
# the pipeline drain tail
TILES = (2048, 2048, 2048, 1024, 512, 512)
BUFS_LOADS = 2
BUFS_ACTS = 3
BUFS_OUT = 2
# PSUM bank budget (8 total)
B_HA = 4
B_HB = 2
B_PO = 2
# merge-tree engine per add: 'v' = vector (DVE), 'p' = gpsimd (Pool)
# adds: l1+=l0, l3+=l2, l5+=l4, l7+=l6, l1+=l3, l5+=l7, cs=l1+l5
# (Pool cannot access PSUM, so it only ever gets these SBUF-only adds)
MERGE_ENGS = "ppppvvv"
# DMA issue engines for the 8 component loads (cycled), the signal load and
# the output store: 's' = sync (SP), 'a' = scalar (Act), 'v' = vector,
# 'p' = gpsimd
COMP_DMA_ENGS = "sasasasa"
SIG_DMA_ENG = "s"
OUT_DMA_ENG = "s"
# 0 = one dma_start per component; N>0 = N dma_starts per tile, each moving
# K/N components via a single 3D access pattern (fewer issues, bigger bursts)
COMP_GROUP_DMA = 0
# engines for the per-layer bias+relu (psum -> sbuf) ops; must be 'v' or 'a'
ENG_H0A = "v"
ENG_H1A = "v"
ENG_H2A = "v"
ENG_OT = "a"
ENG_H0B = "a"
ENG_H1B = "a"
ENG_H2B = "a"
# timing-only: skip all compute, just do the DMA pattern (output is garbage)
SKIP_COMPUTE = False
# repeat whole body via HW loop (timing harness only)
REPS = 1

# bf16 weight-pack column layout ([128, NWB]); lhsT tensors, K on partitions
_C_W0A_SIG = 0      # [128,128] A.T[:, :128]
_C_W0A_CS = 128     # [128,128] Cm.T[:, :128]
_C_W1A_HI = 256     # [128,128] W1.T[:128, :128]
_C_W2A_HI = 384
_C_W3_HI = 512      # [128,128] W3.T[:128, :]
_C_W1A_LO = 640     # [4,128]  W1.T[128:, :128] on partitions 0:4
_C_W2A_LO = 768
_C_W3_LO = 896      # [4,128]  W3.T[128:, :]
_C_W0B_SIG = 1024   # [128,4]  A.T[:, 128:]
_C_W0B_CS = 1028
_C_W1B_HI = 1032    # [128,4]  W1.T[:128, 128:]
_C_W2B_HI = 1036
_C_W1B_LO = 1040    # [4,4]    W1.T[128:, 128:]
_C_W2B_LO = 1044
NWB = 1048
# f32 bias-pack column layout ([128, NWF])
_C_B0A = 0
_C_B1A = 1
_C_B2A = 2
_C_B3 = 3
_C_B0B = 4          # [4,1]
_C_B1B = 5
_C_B2B = 6
NWF = 8


def _build_wpacks(Wm, bm, Wu, bu, W0, b0, W1, b1, W2, b2, W3, b3):
    f8 = np.float64
    Wm, bm, Wu, bu = Wm.astype(f8), bm.astype(f8), Wu.astype(f8), bu.astype(f8)
    W0, b0, W1, b1 = W0.astype(f8), b0.astype(f8), W1.astype(f8), b1.astype(f8)
    W2, b2, W3, b3 = W2.astype(f8), b2.astype(f8), W3.astype(f8), b3.astype(f8)

    A = W0[:, :D]                              # [H, D]
    W0r = W0[:, D:]                            # [H, D]
    Cm = W0r @ (Wu @ Wm)                       # [H, D]
    b0p = b0 + W0r @ (Wu @ (K * bm) + bu)      # [H]

    wb = np.zeros((128, NWB), dtype=np.float64)
    wb[:, _C_W0A_SIG:_C_W0A_SIG + 128] = A.T[:, :128]
    wb[:, _C_W0A_CS:_C_W0A_CS + 128] = Cm.T[:, :128]
    wb[:, _C_W0B_SIG:_C_W0B_SIG + 4] = A.T[:, 128:]
    wb[:, _C_W0B_CS:_C_W0B_CS + 4] = Cm.T[:, 128:]
    for Wx, chi, clo, cbhi, cblo in (
        (W1, _C_W1A_HI, _C_W1A_LO, _C_W1B_HI, _C_W1B_LO),
        (W2, _C_W2A_HI, _C_W2A_LO, _C_W2B_HI, _C_W2B_LO),
    ):
        WT = Wx.T                              # [132 in, 132 out]
        wb[:, chi:chi + 128] = WT[:128, :128]
        wb[:4, clo:clo + 128] = WT[128:, :128]
        wb[:, cbhi:cbhi + 4] = WT[:128, 128:]
        wb[:4, cblo:cblo + 4] = WT[128:, 128:]
    W3T = W3.T                                 # [132, 128]
    wb[:, _C_W3_HI:_C_W3_HI + 128] = W3T[:128, :]
    wb[:4, _C_W3_LO:_C_W3_LO + 128] = W3T[128:, :]

    wf = np.zeros((128, NWF), dtype=np.float64)
    wf[:, _C_B0A] = b0p[:128]
    wf[:, _C_B1A] = b1[:128]
    wf[:, _C_B2A] = b2[:128]
    wf[:, _C_B3] = b3
    wf[:4, _C_B0B] = b0p[128:]
    wf[:4, _C_B1B] = b1[128:]
    wf[:4, _C_B2B] = b2[128:]
    return (np.ascontiguousarray(wb, dtype=NPBF),
            np.ascontiguousarray(wf, dtype=np.float32))


def _trace_kernel(nc: bass.Bass):
    assert sum(TILES) == RB and all(t % FREE == 0 for t in TILES)
    sigT = nc.dram_tensor("sigT", [D, RB], BF16, kind="ExternalInput")
    compT = nc.dram_tensor("compT", [K, D, RB], BF16, kind="ExternalInput")
    wbd = nc.dram_tensor("wb", [128, NWB], BF16, kind="ExternalInput")
    wfd = nc.dram_tensor("wf", [128, NWF], F32, kind="ExternalInput")
    outT = nc.dram_tensor("outT", [D, RB], BF16, kind="ExternalOutput")

    with tile.TileContext(nc) as tc, ExitStack() as ctx:
        wpool = ctx.enter_context(tc.tile_pool(name="weights", bufs=1))
        loads = ctx.enter_context(tc.tile_pool(name="loads", bufs=BUFS_LOADS))
        acts = ctx.enter_context(tc.tile_pool(name="acts", bufs=BUFS_ACTS))
        opool = ctx.enter_context(tc.tile_pool(name="outs", bufs=BUFS_OUT))
        psum = ctx.enter_context(tc.tile_pool(name="psum", bufs=2, space="PSUM"))

        eng = {"s": nc.sync, "a": nc.scalar, "v": nc.vector, "p": nc.gpsimd}

        wbs = wpool.tile([128, NWB], BF16)
        nc.sync.dma_start(wbs[:], wbd.ap())
        wfs = wpool.tile([128, NWF], F32)
        nc.sync.dma_start(wfs[:], wfd.ap())

        def wcol(c, n=128, parts=128):
            return wbs[:parts, c:c + n]

        def bcol(c, parts=128):
            return wfs[:parts, c:c + 1]

        def relu_bias(e, out, ps, bc, parts=128):
            if e == "a":
                nc.scalar.activation(out, ps, ACT.Relu, bias=bcol(bc, parts))
            else:
                eng[e].tensor_scalar(out, ps, bcol(bc, parts), 0.0, ADD, MAX)

        with (tc.For_i(0, REPS, 1) if REPS > 1 else nullcontext()):
            r0 = 0
            for t, TLt in enumerate(TILES):
                sig_v = sigT.ap()[:, r0:r0 + TLt]
                comp_v = [compT.ap()[k, :, r0:r0 + TLt] for k in range(K)]
                out_v = outT.ap()[:, r0:r0 + TLt]
                r0 += TLt

                sig_t = loads.tile([128, TLt], BF16, tag="sig")
                eng[SIG_DMA_ENG].dma_start(sig_t[:], sig_v)

                if COMP_GROUP_DMA:
                    # one SBUF tile [128, K*TLt]; each dma_start moves a
                    # group of components via a 3D AP (k, d, r)
                    gtile = loads.tile([128, K * TLt], BF16, tag="csall",
                                       name="csall")
                    lands = [gtile[:, i * TLt:(i + 1) * TLt] for i in range(K)]
                    gsz = K // COMP_GROUP_DMA
                    for g in range(COMP_GROUP_DMA):
                        src = compT.ap()[g * gsz:(g + 1) * gsz, :,
                                         r0:r0 + TLt].rearrange(
                                             "k d r -> d (k r)")
                        eng[COMP_DMA_ENGS[g % len(COMP_DMA_ENGS)]].dma_start(
                            gtile[:, g * gsz * TLt:(g + 1) * gsz * TLt], src)
                else:
                    lands = [loads.tile([128, TLt], BF16, tag=f"cs{i}",
                                        name=f"cs{i}") for i in range(K)]
                    for i in range(K):
                        eng[COMP_DMA_ENGS[i % len(COMP_DMA_ENGS)]].dma_start(
                            lands[i][:], comp_v[i])
                cs_sum = loads.tile([128, TLt], BF16, tag="cs_sum")
                me = [eng[c] for c in MERGE_ENGS]
                if COMP_GROUP_DMA:
                    # lands are slices of one tile: write tree results into
                    # separate scratch tiles (keeps dependency tracking clean)
                    m01 = loads.tile([128, TLt], BF16, tag="m01", name="m01")
                    m23 = loads.tile([128, TLt], BF16, tag="m23", name="m23")
                    m45 = loads.tile([128, TLt], BF16, tag="m45", name="m45")
                    m67 = loads.tile([128, TLt], BF16, tag="m67", name="m67")
                    me[0].tensor_add(m01[:], lands[0], lands[1])
                    me[1].tensor_add(m23[:], lands[2], lands[3])
                    me[2].tensor_add(m45[:], lands[4], lands[5])
                    me[3].tensor_add(m67[:], lands[6], lands[7])
                    me[4].tensor_add(m01[:], m01[:], m23[:])
                    me[5].tensor_add(m45[:], m45[:], m67[:])
                    me[6].tensor_add(cs_sum[:], m01[:], m45[:])
                else:
                    me[0].tensor_add(lands[1][:], lands[1][:], lands[0][:])
                    me[1].tensor_add(lands[3][:], lands[3][:], lands[2][:])
                    me[2].tensor_add(lands[5][:], lands[5][:], lands[4][:])
                    me[3].tensor_add(lands[7][:], lands[7][:], lands[6][:])
                    me[4].tensor_add(lands[1][:], lands[1][:], lands[3][:])
                    me[5].tensor_add(lands[5][:], lands[5][:], lands[7][:])
                    me[6].tensor_add(cs_sum[:], lands[1][:], lands[5][:])

                out_sb = opool.tile([128, TLt], BF16, tag="out_sb")

                if SKIP_COMPUTE:
                    nc.vector.tensor_copy(out_sb[:], sig_t[:])
                    eng[OUT_DMA_ENG].dma_start(out_v, out_sb[:])
                    continue

                for s in range(TLt // FREE):
                    cols = slice(s * FREE, (s + 1) * FREE)

                    # ---- L0: h0 = relu(A@sigT + Cm@csT + b0') ----
                    ps_h0a = psum.tile([128, FREE], F32, tag="ha", bufs=B_HA)
                    nc.tensor.matmul(ps_h0a[:], wcol(_C_W0A_SIG),
                                     sig_t[:, cols], start=True, stop=False)
                    nc.tensor.matmul(ps_h0a[:], wcol(_C_W0A_CS),
                                     cs_sum[:, cols], start=False, stop=True)
                    ps_h0b = psum.tile([4, FREE], F32, tag="hb", bufs=B_HB)
                    nc.tensor.matmul(ps_h0b[:], wcol(_C_W0B_SIG, 4),
                                     sig_t[:, cols], start=True, stop=False)
                    nc.tensor.matmul(ps_h0b[:], wcol(_C_W0B_CS, 4),
                                     cs_sum[:, cols], start=False, stop=True)
                    h0a = acts.tile([128, FREE], BF16, tag="h0a")
                    relu_bias(ENG_H0A, h0a[:], ps_h0a[:], _C_B0A)
                    h0b = acts.tile([4, FREE], BF16, tag="h0b")
                    relu_bias(ENG_H0B, h0b[:], ps_h0b[:], _C_B0B, parts=4)

                    # ---- L1 ----
                    ps_h1a = psum.tile([128, FREE], F32, tag="ha", bufs=B_HA)
                    nc.tensor.matmul(ps_h1a[:], wcol(_C_W1A_HI),
                                     h0a[:], start=True, stop=False)
                    nc.tensor.matmul(ps_h1a[:], wcol(_C_W1A_LO, 128, parts=4),
                                     h0b[:], start=False, stop=True)
                    ps_h1b = psum.tile([4, FREE], F32, tag="hb", bufs=B_HB)
                    nc.tensor.matmul(ps_h1b[:], wcol(_C_W1B_HI, 4),
                                     h0a[:], start=True, stop=False)
                    nc.tensor.matmul(ps_h1b[:], wcol(_C_W1B_LO, 4, parts=4),
                                     h0b[:], start=False, stop=True)
                    h1a = acts.tile([128, FREE], BF16, tag="h1a")
                    relu_bias(ENG_H1A, h1a[:], ps_h1a[:], _C_B1A)
                    h1b = acts.tile([4, FREE], BF16, tag="h1b")
                    relu_bias(ENG_H1B, h1b[:], ps_h1b[:], _C_B1B, parts=4)

                    # ---- L2 ----
                    ps_h2a = psum.tile([128, FREE], F32, tag="ha", bufs=B_HA)
                    nc.tensor.matmul(ps_h2a[:], wcol(_C_W2A_HI),
                                     h1a[:], start=True, stop=False)
                    nc.tensor.matmul(ps_h2a[:], wcol(_C_W2A_LO, 128, parts=4),
                                     h1b[:], start=False, stop=True)
                    ps_h2b = psum.tile([4, FREE], F32, tag="hb", bufs=B_HB)
                    nc.tensor.matmul(ps_h2b[:], wcol(_C_W2B_HI, 4),
                                     h1a[:], start=True, stop=False)
                    nc.tensor.matmul(ps_h2b[:], wcol(_C_W2B_LO, 4, parts=4),
                                     h1b[:], start=False, stop=True)
                    h2a = acts.tile([128, FREE], BF16, tag="h2a")
                    relu_bias(ENG_H2A, h2a[:], ps_h2a[:], _C_B2A)
                    h2b = acts.tile([4, FREE], BF16, tag="h2b")
                    relu_bias(ENG_H2B, h2b[:], ps_h2b[:], _C_B2B, parts=4)

                    # ---- L3: outT = W3 @ h2 + b3 (feature-major) ----
                    ps_o = psum.tile([128, FREE], F32, tag="po", bufs=B_PO)
                    nc.tensor.matmul(ps_o[:], wcol(_C_W3_HI),
                                     h2a[:], start=True, stop=False)
                    nc.tensor.matmul(ps_o[:], wcol(_C_W3_LO, 128, parts=4),
                                     h2b[:], start=False, stop=True)
                    eng[ENG_OT].activation(out_sb[:, cols], ps_o[:],
                                           ACT.Identity, bias=bcol(_C_B3))

                eng[OUT_DMA_ENG].dma_start(out_v, out_sb[:])

    return nc


_CACHED_NC = None


def _get_nc():
    global _CACHED_NC
    if _CACHED_NC is None:
        nc = bacc.Bacc("TRN2", target_bir_lowering=False, debug=False,
                       enable_asserts=False, num_devices=NCORES)
        _trace_kernel(nc)
        nc.compile()
        _CACHED_NC = nc
    return _CACHED_NC


def make_in_maps(inputs):
    """Host staging: shard over B, transpose to feature-major, cast to bf16."""
    signal = np.asarray(inputs["signal"], dtype=np.float32)
    components = np.asarray(inputs["components"], dtype=np.float32)
    wb, wf = _build_wpacks(*[np.asarray(inputs[k], dtype=np.float32) for k in
                             ("Wm", "bm", "Wu", "bu", "W0", "b0",
                              "W1", "b1", "W2", "b2", "W3", "b3")])
    sigT = signal.astype(NPBF).T                      # [D, B] view
    compT = components.astype(NPBF).transpose(0, 2, 1)  # [K, D, B] view
    in_maps = []
    for c in range(NCORES):
        r0 = c * RB
        in_maps.append({
            "sigT": np.ascontiguousarray(sigT[:, r0:r0 + RB]),
            "compT": np.ascontiguousarray(compT[:, :, r0:r0 + RB]),
            "wb": wb,
            "wf": wf,
        })
    return in_maps


def kernel(**inputs):
    nc = _get_nc()
    res = bass_utils.run_bass_kernel_spmd(nc, make_in_maps(inputs),
                                          core_ids=list(range(NCORES)))
    out = np.concatenate(
        [res.results[c]["outT"].astype(np.float32).T for c in range(NCORES)],
        axis=0)
    return np.ascontiguousarray(out)
